# revision 61
# baseline (speedup 1.0000x reference)
"""Trainium2 Bass kernel for CausalSelfAttention (B=4, T=2048, C=1024, H=16)
with additive prev-prob key bias.

Sharding: 8 cores = data-parallel over B (4) x tensor-parallel over head
halves (2).  Each core computes qkv for its 8 heads, causal attention, and a
partial output projection (row-parallel W_proj); host sums the two partials
per batch at unshard time.

Per-core device algorithm (HAM-warm bf16 schedule):
  - All matmuls run bf16 x bf16 with fp32 PSUM accumulation (rel err
    ~4e-3 vs the 2e-2 gate).  K^T and Q^T are kept feature-major with
    head pairs stacked in the 128 partitions, so QK^T runs as two
    row-tiled (K=64) concurrent matmuls per issue slot.
  - Scores are computed transposed (keys on partitions).  The additive
    prev-prob bias folds into the exp as a per-partition bias AP
    (exp(qk/8 + adj[k])); the softmax denominator comes from a constant
    ones column appended to V (M=65 PV matmuls).
  - Causality: block-trimmed matmul widths + one fused 128x(2x128)
    triangular mask multiply per diagonal block.
  - Normalization is fully deferred: each head pair only evacuates y^T;
    2+ pairs later a filler chain bridges the denominator rows to
    partition 0 (tiny DMA), reciprocal_approx_fast (DVE, zero-wait),
    partition_broadcast (gpsimd ucode — the ONLY custom gpsimd op, so
    its library stays resident), and base-0 stack muls (DVE).  No engine
    FIFO ever blocks on DMA latency, so HAM stays at K=8/8.
  - Startup: warmup matmuls un-throttle the PE during the initial DMAs;
    weight blocks are split across both HWDGE queues (sync: Wq+Wv+Wproj,
    scalar: x(0)+Wk) so every block lands just before its first use; the
    adj bias is pre-shuffled on the host to avoid a gather DMA.
  - Queue topology: x loads ride gpsimd SWDGE (chunks 1+), projection
    output DMAs alternate sync/gpsimd, evacuations stay on DVE so the
    scalar engine runs pure exp (its ~165us is the #2 engine floor).
"""

import math
from contextlib import ExitStack

import numpy as np

import concourse.bass as bass
import concourse.tile as tile
from concourse import bacc, mybir

F32 = mybir.dt.float32
F32R = mybir.dt.float32r
BF16 = mybir.dt.bfloat16

USE_F32R = True
MMD = F32R if USE_F32R else F32

B, T, C, H = 4, 2048, 1024, 16
HD = C // H          # 64
NCORES = 8
HPC = H // 2         # 8 heads per core
FPC = HPC * HD       # 512 features per core
NKT = T // 128       # 16 key tiles
NQC = T // 512       # 4 query chunks (also the x t-chunks)
NCT = C // 128       # 8 contraction tiles
EPS_BIAS = 0.1
SCALE = 1.0 / math.sqrt(HD)


def build(tc, out_ap, xT, wqkv, wproj, adj, tri_dram):
    """Emit the per-core kernel into TileContext tc.

    out_ap : (T, C)    partial projection output (needs pair-sum on host)
    xT     : (C, T)    x[b] transposed
    wqkv   : (C, 3*FPC) [Wq_g | Wk_g | Wv_g] columns for this head group
    wproj  : (FPC, C)  W_proj rows for this head group
    adj    : (T,)      -EPS_BIAS * log(prev_probs[b] + 1e-10)
    tri_dram: (128,128) upper-triangular ones (tri[k,q] = 1 iff k <= q)
    """
    nc = tc.nc
    ctx = tc.ctx
    Exp = mybir.ActivationFunctionType.Exp

    const = ctx.enter_context(tc.tile_pool(name="const", bufs=1))
    xs_pool = ctx.enter_context(tc.tile_pool(name="xs", bufs=2))
    qt_pool = ctx.enter_context(tc.tile_pool(name="qt", bufs=10))
    se_pool = ctx.enter_context(tc.tile_pool(name="se", bufs=6))
    tmp_pool = ctx.enter_context(tc.tile_pool(name="tmp", bufs=8))
    stkb_pool = ctx.enter_context(tc.tile_pool(name="stkb", bufs=3))
    scale_pool = ctx.enter_context(tc.tile_pool(name="scale", bufs=4))
    stack_pool = ctx.enter_context(tc.tile_pool(name="stack", bufs=8))
    pout_pool = ctx.enter_context(tc.tile_pool(name="pout", bufs=4))

    ps_pool = ctx.enter_context(tc.tile_pool(name="ps", bufs=2, space="PSUM"))
    st_pool = ctx.enter_context(tc.tile_pool(name="st", bufs=2, space="PSUM"))
    y_pool = ctx.enter_context(tc.tile_pool(name="y", bufs=2, space="PSUM"))

    # ---- PE warmup: drive HAM to K=8/8 while the initial DMAs stream ----
    warm_w = const.tile([128, 128], F32, name="warm_w")
    nc.vector.memset(warm_w, 0.25)
    wps = ps_pool.tile([128, 512], F32, tag="ps", name="warm_ps")
    for i in range(40):
        nc.tensor.matmul(wps[:, 0:128], warm_w, warm_w, start=True, stop=True)

    # ---- persistent buffers (loads emitted after chunk-0 x loads) ----
    wq_sb = const.tile([128, NCT, 3 * FPC], BF16, name="wq_sb")     # 24KB/p
    wp_sb = const.tile([128, FPC // 128, C], BF16, name="wp_sb")    # 8KB/p
    kt = const.tile([128, HPC // 2, T], BF16, name="kt")            # 16KB/p
    # constant ones column -> PV accumulates the softmax denominator
    v2 = const.tile([128, NKT, HPC, HD + 1], BF16, name="v2")       # 16.6KB/p
    nc.vector.memset(v2[:, :, :, HD:HD + 1], 1.0)
    adj_sb = const.tile([128, NKT], F32, name="adj_sb")
    dn = const.tile([1, 1024], F32, name="dn")  # den rows bridged to part 0
    tri = const.tile([128, 128], F32, name="tri")
    tri_bf = const.tile([128, 128], BF16, name="tri_bf")
    tri_b = tri_bf.unsqueeze(1).to_broadcast([128, 2, 128])

    def load_weights():
        """Queue plan: sync = Wq + Wv + Wproj, scalar = x(0) (already
        queued) + Wk + adj + tri.  One DMA per weight block — small DMAs
        are dispatch/credit-limited (~1.4us each), not bandwidth-limited."""
        wqkv3 = wqkv.rearrange("(c p) f -> p c f", p=128)
        for blk, eng in ((0, nc.sync), (2, nc.sync), (1, nc.scalar)):
            eng.dma_start(
                out=wq_sb[:, :, blk * FPC:(blk + 1) * FPC],
                in_=wqkv3[:, :, blk * FPC:(blk + 1) * FPC],
            )
        nc.scalar.dma_start(out=adj_sb, in_=adj[:, :])
        nc.scalar.dma_start(out=tri, in_=tri_dram[:, :])
        nc.vector.tensor_copy(tri_bf, tri)
        nc.sync.dma_start(
            out=wp_sb, in_=wproj.rearrange("(i p) c -> p i c", p=128)
        )

    qts_store = {}
    stacks_store = {}
    tmps_store = {}

    def gen_chunk(qc):
        """Emit one t-chunk's pre-attention work as resumable items:
        x loads, JIT Q^T, K^T tiles, V tiles.  x loads alternate between the
        two HWDGE queues, and spacer yields separate them from the first
        matmul group so the PE FIFO never waits on an in-flight load."""
        xs_eng = nc.scalar if qc <= 1 else nc.gpsimd
        xsc = xs_pool.tile([128, NCT, 512], BF16, tag="xs", name=f"xs_{qc}")
        xs_eng.dma_start(
            out=xsc,
            in_=xT[:, qc * 512:(qc + 1) * 512].rearrange(
                "(c p) q -> p c q", p=128),
        )
        xs_tiles = [xsc[:, c, :] for c in range(NCT)]
        for _ in range(7 if qc > 0 else 1):
            yield
        qts = []
        for p in range(HPC // 2):
            ps = ps_pool.tile([128, 512], F32, tag="ps", name=f"qps_{qc}_{p}")
            for c in range(NCT):
                nc.tensor.matmul(
                    ps,
                    wq_sb[:, c, p * 128:(p + 1) * 128],
                    xs_tiles[c],
                    start=(c == 0),
                    stop=(c == NCT - 1),
                )
            qt = qt_pool.tile([128, 512], BF16, tag="qt", name=f"qt_{qc}_{p}")
            nc.vector.tensor_copy(qt, ps)
            qts.append(qt)
            yield
        qts_store[qc] = qts
        for p in range(HPC // 2):
            ps = ps_pool.tile([128, 512], F32, tag="ps", name=f"kps_{qc}_{p}")
            for c in range(NCT):
                nc.tensor.matmul(
                    ps,
                    wq_sb[:, c, FPC + p * 128:FPC + (p + 1) * 128],
                    xs_tiles[c],
                    start=(c == 0),
                    stop=(c == NCT - 1),
                )
            nc.vector.tensor_copy(kt[:, p, qc * 512:(qc + 1) * 512], ps)
            yield
        for j in range(4):
            kt_i = qc * 4 + j
            ps = ps_pool.tile([128, 512], F32, tag="ps", name=f"vps_{qc}_{j}")
            for c in range(NCT):
                nc.tensor.matmul(
                    ps,
                    xs_tiles[c][:, j * 128:(j + 1) * 128],
                    wq_sb[:, c, 2 * FPC:3 * FPC],
                    start=(c == 0),
                    stop=(c == NCT - 1),
                )
            nc.vector.tensor_copy(
                v2[:, kt_i, :, 0:HD],
                ps.rearrange("p (h d) -> p h d", h=HPC),
            )
            yield

    def gen_norm(qc, p):
        """Deferred denominator chain for head pair (qc, p): DMA-bridge the
        den rows (partition 64) to partition 0, reciprocal there (DVE),
        partition-broadcast to rows 0..63 (gpsimd, SBUF-to-SBUF), base-0
        stack muls (gpsimd).  Pulled 2+ pairs after the pair finished, so
        nothing ever stalls."""
        tmpA, tmpB = tmps_store[(qc, p)]
        nc.sync.dma_start(out=dn[0:1, 0:512], in_=tmpA[HD:HD + 1, :])
        nc.sync.dma_start(out=dn[0:1, 512:1024], in_=tmpB[HD:HD + 1, :])
        yield
        sc = scale_pool.tile([64, 1024], F32, tag="scale", name=f"sc_{qc}_{p}")
        nc.vector.reciprocal_approx_fast(sc[0:1, :], dn[0:1, :])
        yield
        nc.gpsimd.partition_broadcast(sc[0:64, 0:512], sc[0:1, 0:512],
                                      channels=64)
        nc.gpsimd.partition_broadcast(sc[0:64, 512:1024], sc[0:1, 512:1024],
                                      channels=64)
        yield
        stack = stack_pool.tile([128, 512], BF16, tag="stack", name=f"stk_{qc}_{p}")
        nc.vector.tensor_mul(stack[0:64, :], tmpA[0:64, :], sc[0:64, 0:512])
        yield
        stkB = stkb_pool.tile([64, 512], BF16, tag="stkB", name=f"skB_{qc}_{p}")
        nc.vector.tensor_mul(stkB[0:64, :], tmpB[0:64, :], sc[0:64, 512:1024])
        nc.sync.dma_start(out=stack[64:128, :], in_=stkB[0:64, :])
        stacks_store[qc][p] = stack
        yield

    def gen_proj(qc):
        stacks = stacks_store[qc]
        for tq in range(4):
            row0 = qc * 512 + tq * 128
            for ch in range(2):
                ps = ps_pool.tile([128, 512], F32, tag="ps",
                                  name=f"pps_{qc}_{tq}_{ch}")
                for p in range(HPC // 2):
                    nc.tensor.matmul(
                        ps,
                        stacks[p][:, tq * 128:(tq + 1) * 128],
                        wp_sb[:, p, ch * 512:(ch + 1) * 512],
                        start=(p == 0),
                        stop=(p == HPC // 2 - 1),
                    )
                pout = pout_pool.tile([128, 512], F32, tag="pout",
                                      name=f"po_{qc}_{tq}_{ch}")
                nc.vector.tensor_copy(pout, ps)
                # alternate DMA queue to break the pout-copy <-> out-DMA
                # recycle round-robin
                dma_eng = nc.sync if ch == 0 else nc.gpsimd
                dma_eng.dma_start(
                    out=out_ap[row0:row0 + 128, ch * 512:(ch + 1) * 512],
                    in_=pout,
                )
                yield

    def emit_tail_proj():
        """Last chunk's projection: all 8 groups' p=0..2 accumulations are
        emitted FIRST (they only need stacks 0..2, so they overlap the final
        norm chain on the PE), then the p=3 matmuls, then the evacuations.
        Eight PSUM banks: 2 ps + 2 y + halves of 2 st tiles."""
        qc = NQC - 1
        stacks = stacks_store[qc]
        groups = []
        st_tiles = []
        for tq in range(4):
            for ch in range(2):
                gi = tq * 2 + ch
                if gi < 2:
                    ps = ps_pool.tile([128, 512], F32, tag="ps",
                                      name=f"tp_ps_{gi}")
                elif gi < 4:
                    ps = y_pool.tile([128, 512], F32, tag="y",
                                     name=f"tp_y_{gi}")
                else:
                    if gi % 2 == 0:
                        st_tiles.append(st_pool.tile(
                            [128, 1024], F32, tag="st", name=f"tp_st_{gi}"))
                    half = st_tiles[-1]
                    ps = half[:, 0:512] if gi % 2 == 0 else half[:, 512:1024]
                for p in range(3):
                    nc.tensor.matmul(
                        ps,
                        stacks[p][:, tq * 128:(tq + 1) * 128],
                        wp_sb[:, p, ch * 512:(ch + 1) * 512],
                        start=(p == 0),
                        stop=False,
                    )
                groups.append((ps, tq, ch))
        for ps, tq, ch in groups:
            nc.tensor.matmul(
                ps,
                stacks[3][:, tq * 128:(tq + 1) * 128],
                wp_sb[:, 3, ch * 512:(ch + 1) * 512],
                start=False,
                stop=True,
            )
        for gi, (ps, tq, ch) in enumerate(groups):
            row0 = qc * 512 + tq * 128
            pout = pout_pool.tile([128, 512], F32, tag="pout",
                                  name=f"tp_po_{tq}_{ch}")
            if gi % 2 == 0:
                nc.vector.tensor_copy(pout, ps)
            else:
                nc.scalar.copy(pout, ps)
            dma_eng = nc.sync if ch == 0 else nc.gpsimd
            dma_eng.dma_start(
                out=out_ap[row0:row0 + 128, ch * 512:(ch + 1) * 512],
                in_=pout,
            )

    gen0 = gen_chunk(0)
    next(gen0)          # x(0) loads onto the scalar queue first
    load_weights()      # then the weight blocks, split across both queues
    for _ in gen0:
        pass

    fillers = []

    def pull(n):
        for _ in range(n):
            while fillers:
                try:
                    next(fillers[0])
                    break
                except StopIteration:
                    fillers.pop(0)

    for qc in range(NQC):
        stacks_store[qc] = [None] * 4
        if qc > 0:
            fillers.append(gen_norm(qc - 1, 2))
            fillers.append(gen_norm(qc - 1, 3))
        if qc + 1 < NQC:
            fillers.append(gen_chunk(qc + 1))
        if qc > 0:
            fillers.append(gen_proj(qc - 1))

        # ---- attention for this query chunk, per head pair ----
        nki = 4 * qc + 4
        qts = qts_store[qc]
        for p in range(HPC // 2):
            qt = qts[p]
            yA = y_pool.tile([128, 512], F32, tag="y", name=f"yA_{qc}_{p}")
            yB = y_pool.tile([128, 512], F32, tag="y", name=f"yB_{qc}_{p}")
            for ki in range(nki):
                r = ki - 4 * qc  # >= 0 on the block diagonal
                n0 = 128 * r if r > 0 else 0
                st = st_pool.tile([128, 1024], F32, tag="st",
                                  name=f"st_{qc}_{p}_{ki}")
                st3 = st.rearrange("p (h q) -> p h q", h=2)
                kslice = slice(ki * 128, (ki + 1) * 128)
                nc.tensor.matmul(
                    st3[:, 0, n0:512], kt[0:64, p, kslice], qt[0:64, n0:512],
                    start=True, stop=True,
                )
                nc.tensor.matmul(
                    st3[:, 1, n0:512], kt[64:128, p, kslice], qt[64:128, n0:512],
                    start=True, stop=True,
                )
                se = se_pool.tile([128, 1024], BF16, tag="se",
                                  name=f"se_{qc}_{p}_{ki}")
                se3 = se.rearrange("p (h q) -> p h q", h=2)
                nc.scalar.activation(
                    se3[:, :, n0:512], st3[:, :, n0:512], Exp,
                    bias=adj_sb[:, ki:ki + 1], scale=SCALE,
                )
                if r >= 0:
                    nc.vector.tensor_mul(
                        se3[:, :, n0:n0 + 128], se3[:, :, n0:n0 + 128], tri_b
                    )
                nc.tensor.matmul(
                    yA[0:HD + 1, n0:512], v2[:, ki, 2 * p, :], se3[:, 0, n0:512],
                    start=(ki == 0), stop=(ki == nki - 1), skip_group_check=True,
                )
                nc.tensor.matmul(
                    yB[0:HD + 1, n0:512], v2[:, ki, 2 * p + 1, :], se3[:, 1, n0:512],
                    start=(ki == 0), stop=(ki == nki - 1), skip_group_check=True,
                )
                # cadence: spread the filler items over the chunk's ki slots
                if qc == 0:
                    pull(2)
                elif qc == 1:
                    pull(2 if ki % 2 == 0 else 1)
                elif qc == 2:
                    pull(1)
                else:
                    # reserve fillers for the ACT-bound back half of the
                    # last chunk, where the PE otherwise runs dry
                    pull(1 if (ki >= 8 and ki % 2 == 0) else 0)

            # evacuate y^T (+ row HD denominators); everything else is
            # deferred to gen_norm pulled >= 2 pairs later
            tmpA = tmp_pool.tile([65, 512], F32, tag="tmp", name=f"tmpA_{qc}_{p}")
            nc.vector.tensor_copy(tmpA[0:HD + 1, :], yA[0:HD + 1, :])
            tmpB = tmp_pool.tile([65, 512], F32, tag="tmp", name=f"tmpB_{qc}_{p}")
            nc.vector.tensor_copy(tmpB[0:HD + 1, :], yB[0:HD + 1, :])
            tmps_store[(qc, p)] = (tmpA, tmpB)
            if p < 2 or qc == NQC - 1:
                if qc == NQC - 1:
                    # jump the queue: the tail projection needs these
                    # stacks the moment attention ends
                    fillers.insert(0, gen_norm(qc, p))
                else:
                    fillers.append(gen_norm(qc, p))
            pull(2)

        pull(1000)

    # ---- tail: last chunk's projection ----
    emit_tail_proj()


def make_nc():
    nc = bacc.Bacc("TRN2", target_bir_lowering=False, debug=False,
                   num_devices=NCORES)
    xT = nc.dram_tensor("xT", [C, T], BF16, kind="ExternalInput")
    wqkv = nc.dram_tensor("wqkv", [C, 3 * FPC], BF16, kind="ExternalInput")
    wproj = nc.dram_tensor("wproj", [FPC, C], BF16, kind="ExternalInput")
    adj = nc.dram_tensor("adj", [128, NKT], F32, kind="ExternalInput")
    out = nc.dram_tensor("out", [T, C], F32, kind="ExternalOutput")
    tri_np = np.triu(np.ones((128, 128), dtype=np.float32))
    tri_dram = nc.inline_tensor(tri_np, name="tri_const")
    with ExitStack() as ctx:
        tc = ctx.enter_context(tile.TileContext(nc))
        tc.ctx = ctx
        build(tc, out[:, :], xT[:, :], wqkv[:, :], wproj[:, :], adj[:],
              tri_dram)
    nc.compile()
    return nc


def shard_inputs(x, prev_probs, W_attn, W_proj):
    import ml_dtypes

    in_maps = []
    for core in range(NCORES):
        b, g = divmod(core, 2)
        xT = np.ascontiguousarray(x[b].T)
        wq = W_attn[:, g * FPC:(g + 1) * FPC]
        wk = W_attn[:, C + g * FPC:C + (g + 1) * FPC]
        wv = W_attn[:, 2 * C + g * FPC:2 * C + (g + 1) * FPC]
        wqkv = np.ascontiguousarray(np.concatenate([wq, wk, wv], axis=1))
        wproj = np.ascontiguousarray(W_proj[g * FPC:(g + 1) * FPC, :])
        adj = (-np.float32(EPS_BIAS)
               * np.log(prev_probs[b] + np.float32(1e-10))).astype(np.float32)
        # pre-shuffle to adj_sb layout: [key-within-tile (partition), tile]
        adj = np.ascontiguousarray(adj.reshape(NKT, 128).T)
        in_maps.append(
            {
                "xT": xT.astype(ml_dtypes.bfloat16),
                "wqkv": wqkv.astype(ml_dtypes.bfloat16),
                "wproj": wproj.astype(ml_dtypes.bfloat16),
                "adj": adj,
            }
        )
    return in_maps


_CACHED_NC = None


def kernel(x, prev_probs, W_attn, W_proj, trace=False, tmpdir=None):
    global _CACHED_NC
    from concourse.bass_utils import run_bass_kernel_spmd

    x = np.asarray(x, dtype=np.float32)
    prev_probs = np.asarray(prev_probs, dtype=np.float32)
    W_attn = np.asarray(W_attn, dtype=np.float32)
    W_proj = np.asarray(W_proj, dtype=np.float32)

    if _CACHED_NC is None:
        _CACHED_NC = make_nc()
    nc = _CACHED_NC

    in_maps = shard_inputs(x, prev_probs, W_attn, W_proj)
    res = run_bass_kernel_spmd(
        nc, in_maps, core_ids=list(range(NCORES)), trace=trace, tmpdir=tmpdir
    )
    parts = [r["out"] for r in res.results]
    out = np.empty((B, T, C), dtype=np.float32)
    for b in range(B):
        out[b] = parts[2 * b] + parts[2 * b + 1]
    kernel.last_results = res
    return out


# revision 62
# speedup vs baseline: 1.0266x; 1.0266x over previous
"""Trainium2 Bass kernel for CausalSelfAttention (B=4, T=2048, C=1024, H=16)
with additive prev-prob key bias.

Sharding: 8 cores = data-parallel over B (4) x tensor-parallel over head
halves (2).  Each core computes qkv for its 8 heads, causal attention, and a
partial output projection (row-parallel W_proj); host sums the two partials
per batch at unshard time.

Per-core device algorithm (HAM-warm bf16 schedule):
  - All matmuls run bf16 x bf16 with fp32 PSUM accumulation (rel err
    ~4e-3 vs the 2e-2 gate).  K^T and Q^T are kept feature-major with
    head pairs stacked in the 128 partitions, so QK^T runs as two
    row-tiled (K=64) concurrent matmuls per issue slot.
  - Scores are computed transposed (keys on partitions).  The additive
    prev-prob bias folds into the exp as a per-partition bias AP
    (exp(qk/8 + adj[k])); the softmax denominator comes from a constant
    ones column appended to V (M=65 PV matmuls).
  - Causality: block-trimmed matmul widths + one fused 128x(2x128)
    triangular mask multiply per diagonal block.
  - Normalization is fully deferred: each head pair only evacuates y^T;
    2+ pairs later a filler chain bridges the denominator rows to
    partition 0 (tiny DMA), reciprocal_approx_fast (DVE, zero-wait),
    partition_broadcast (gpsimd ucode — the ONLY custom gpsimd op, so
    its library stays resident), and base-0 stack muls (DVE).  No engine
    FIFO ever blocks on DMA latency, so HAM stays at K=8/8.
  - Startup: warmup matmuls un-throttle the PE during the initial DMAs;
    weight blocks are split across both HWDGE queues (sync: Wq+Wv+Wproj,
    scalar: x(0)+Wk) so every block lands just before its first use; the
    adj bias is pre-shuffled on the host to avoid a gather DMA.
  - Queue topology: x loads ride gpsimd SWDGE (chunks 1+), projection
    output DMAs alternate sync/gpsimd, evacuations stay on DVE so the
    scalar engine runs pure exp (its ~165us is the #2 engine floor).
"""

import math
from contextlib import ExitStack

import numpy as np

import concourse.bass as bass
import concourse.tile as tile
from concourse import bacc, mybir

F32 = mybir.dt.float32
F32R = mybir.dt.float32r
BF16 = mybir.dt.bfloat16

USE_F32R = True
MMD = F32R if USE_F32R else F32

B, T, C, H = 4, 2048, 1024, 16
HD = C // H          # 64
NCORES = 8
HPC = H // 2         # 8 heads per core
FPC = HPC * HD       # 512 features per core
NKT = T // 128       # 16 key tiles
NQC = T // 512       # 4 query chunks (also the x t-chunks)
NCT = C // 128       # 8 contraction tiles
EPS_BIAS = 0.1
SCALE = 1.0 / math.sqrt(HD)


def build(tc, out_ap, xT, wqkv, wproj, adj, tri_dram):
    """Emit the per-core kernel into TileContext tc.

    out_ap : (T, C)    partial projection output (needs pair-sum on host)
    xT     : (C, T)    x[b] transposed
    wqkv   : (C, 3*FPC) [Wq_g | Wk_g | Wv_g] columns for this head group
    wproj  : (FPC, C)  W_proj rows for this head group
    adj    : (T,)      -EPS_BIAS * log(prev_probs[b] + 1e-10)
    tri_dram: (128,128) upper-triangular ones (tri[k,q] = 1 iff k <= q)
    """
    nc = tc.nc
    ctx = tc.ctx
    Exp = mybir.ActivationFunctionType.Exp

    const = ctx.enter_context(tc.tile_pool(name="const", bufs=1))
    xs_pool = ctx.enter_context(tc.tile_pool(name="xs", bufs=2))
    qt_pool = ctx.enter_context(tc.tile_pool(name="qt", bufs=10))
    se_pool = ctx.enter_context(tc.tile_pool(name="se", bufs=6))
    tmp_pool = ctx.enter_context(tc.tile_pool(name="tmp", bufs=8))
    stkb_pool = ctx.enter_context(tc.tile_pool(name="stkb", bufs=3))
    scale_pool = ctx.enter_context(tc.tile_pool(name="scale", bufs=4))
    stack_pool = ctx.enter_context(tc.tile_pool(name="stack", bufs=8))
    pout_pool = ctx.enter_context(tc.tile_pool(name="pout", bufs=4))

    ps_pool = ctx.enter_context(tc.tile_pool(name="ps", bufs=2, space="PSUM"))
    st_pool = ctx.enter_context(tc.tile_pool(name="st", bufs=2, space="PSUM"))
    y_pool = ctx.enter_context(tc.tile_pool(name="y", bufs=2, space="PSUM"))

    # ---- PE warmup: drive HAM to K=8/8 while the initial DMAs stream ----
    warm_w = const.tile([128, 128], F32, name="warm_w")
    nc.vector.memset(warm_w, 0.25)
    wps = ps_pool.tile([128, 512], F32, tag="ps", name="warm_ps")
    for i in range(40):
        nc.tensor.matmul(wps[:, 0:128], warm_w, warm_w, start=True, stop=True)

    # ---- persistent buffers (loads emitted after chunk-0 x loads) ----
    wq_sb = const.tile([128, NCT, 3 * FPC], BF16, name="wq_sb")     # 24KB/p
    wp_sb = const.tile([128, FPC // 128, C], BF16, name="wp_sb")    # 8KB/p
    kt = const.tile([128, HPC // 2, T], BF16, name="kt")            # 16KB/p
    # constant ones column -> PV accumulates the softmax denominator
    v2 = const.tile([128, NKT, HPC, HD + 1], BF16, name="v2")       # 16.6KB/p
    nc.vector.memset(v2[:, :, :, HD:HD + 1], 1.0)
    adj_sb = const.tile([128, NKT], F32, name="adj_sb")
    dn = const.tile([1, 1024], F32, name="dn")  # den rows bridged to part 0
    tri = const.tile([128, 128], F32, name="tri")
    tri_bf = const.tile([128, 128], BF16, name="tri_bf")
    tri_b = tri_bf.unsqueeze(1).to_broadcast([128, 2, 128])

    def load_weights():
        """Queue plan: sync = Wq + Wv + Wproj, scalar = x(0) (already
        queued) + Wk + adj + tri.  One DMA per weight block — small DMAs
        are dispatch/credit-limited (~1.4us each), not bandwidth-limited."""
        wqkv3 = wqkv.rearrange("(c p) f -> p c f", p=128)
        for blk, eng in ((0, nc.sync), (2, nc.sync), (1, nc.scalar)):
            eng.dma_start(
                out=wq_sb[:, :, blk * FPC:(blk + 1) * FPC],
                in_=wqkv3[:, :, blk * FPC:(blk + 1) * FPC],
            )
        nc.scalar.dma_start(out=adj_sb, in_=adj[:, :])
        nc.scalar.dma_start(out=tri, in_=tri_dram[:, :])
        nc.vector.tensor_copy(tri_bf, tri)
        nc.sync.dma_start(
            out=wp_sb, in_=wproj.rearrange("(i p) c -> p i c", p=128)
        )

    qts_store = {}
    stacks_store = {}
    tmps_store = {}

    def gen_chunk(qc):
        """Emit one t-chunk's pre-attention work as resumable items:
        x loads, JIT Q^T, K^T tiles, V tiles.  x loads alternate between the
        two HWDGE queues, and spacer yields separate them from the first
        matmul group so the PE FIFO never waits on an in-flight load."""
        xs_eng = nc.scalar if qc <= 1 else nc.gpsimd
        xsc = xs_pool.tile([128, NCT, 512], BF16, tag="xs", name=f"xs_{qc}")
        xs_eng.dma_start(
            out=xsc,
            in_=xT[:, qc * 512:(qc + 1) * 512].rearrange(
                "(c p) q -> p c q", p=128),
        )
        xs_tiles = [xsc[:, c, :] for c in range(NCT)]
        for _ in range(7 if qc > 0 else 1):
            yield
        qts = []
        for p in range(HPC // 2):
            ps = ps_pool.tile([128, 512], F32, tag="ps", name=f"qps_{qc}_{p}")
            for c in range(NCT):
                nc.tensor.matmul(
                    ps,
                    wq_sb[:, c, p * 128:(p + 1) * 128],
                    xs_tiles[c],
                    start=(c == 0),
                    stop=(c == NCT - 1),
                )
            qt = qt_pool.tile([128, 512], BF16, tag="qt", name=f"qt_{qc}_{p}")
            nc.vector.tensor_copy(qt, ps)
            qts.append(qt)
            yield
        qts_store[qc] = qts
        for p in range(HPC // 2):
            ps = ps_pool.tile([128, 512], F32, tag="ps", name=f"kps_{qc}_{p}")
            for c in range(NCT):
                nc.tensor.matmul(
                    ps,
                    wq_sb[:, c, FPC + p * 128:FPC + (p + 1) * 128],
                    xs_tiles[c],
                    start=(c == 0),
                    stop=(c == NCT - 1),
                )
            nc.vector.tensor_copy(kt[:, p, qc * 512:(qc + 1) * 512], ps)
            yield
        for j in range(4):
            kt_i = qc * 4 + j
            ps = ps_pool.tile([128, 512], F32, tag="ps", name=f"vps_{qc}_{j}")
            for c in range(NCT):
                nc.tensor.matmul(
                    ps,
                    xs_tiles[c][:, j * 128:(j + 1) * 128],
                    wq_sb[:, c, 2 * FPC:3 * FPC],
                    start=(c == 0),
                    stop=(c == NCT - 1),
                )
            nc.vector.tensor_copy(
                v2[:, kt_i, :, 0:HD],
                ps.rearrange("p (h d) -> p h d", h=HPC),
            )
            yield

    def gen_norm(qc, p):
        """Deferred denominator chain for head pair (qc, p): DMA-bridge the
        den rows (partition 64) to partition 0, reciprocal there (DVE),
        partition-broadcast to rows 0..63 (gpsimd, SBUF-to-SBUF), base-0
        stack muls (gpsimd).  Pulled 2+ pairs after the pair finished, so
        nothing ever stalls."""
        tmpA, tmpB = tmps_store[(qc, p)]
        nc.sync.dma_start(out=dn[0:1, 0:512], in_=tmpA[HD:HD + 1, :])
        nc.sync.dma_start(out=dn[0:1, 512:1024], in_=tmpB[HD:HD + 1, :])
        yield
        sc = scale_pool.tile([64, 1024], F32, tag="scale", name=f"sc_{qc}_{p}")
        nc.vector.reciprocal_approx_fast(sc[0:1, :], dn[0:1, :])
        yield
        nc.gpsimd.partition_broadcast(sc[0:64, 0:512], sc[0:1, 0:512],
                                      channels=64)
        nc.gpsimd.partition_broadcast(sc[0:64, 512:1024], sc[0:1, 512:1024],
                                      channels=64)
        yield
        stack = stack_pool.tile([128, 512], BF16, tag="stack", name=f"stk_{qc}_{p}")
        nc.vector.tensor_mul(stack[0:64, :], tmpA[0:64, :], sc[0:64, 0:512])
        yield
        stkB = stkb_pool.tile([64, 512], BF16, tag="stkB", name=f"skB_{qc}_{p}")
        nc.vector.tensor_mul(stkB[0:64, :], tmpB[0:64, :], sc[0:64, 512:1024])
        nc.sync.dma_start(out=stack[64:128, :], in_=stkB[0:64, :])
        stacks_store[qc][p] = stack
        yield

    def gen_proj(qc):
        stacks = stacks_store[qc]
        for tq in range(4):
            row0 = qc * 512 + tq * 128
            for ch in range(2):
                ps = ps_pool.tile([128, 512], F32, tag="ps",
                                  name=f"pps_{qc}_{tq}_{ch}")
                for p in range(HPC // 2):
                    nc.tensor.matmul(
                        ps,
                        stacks[p][:, tq * 128:(tq + 1) * 128],
                        wp_sb[:, p, ch * 512:(ch + 1) * 512],
                        start=(p == 0),
                        stop=(p == HPC // 2 - 1),
                    )
                pout = pout_pool.tile([128, 512], F32, tag="pout",
                                      name=f"po_{qc}_{tq}_{ch}")
                nc.vector.tensor_copy(pout, ps)
                # alternate DMA queue to break the pout-copy <-> out-DMA
                # recycle round-robin
                dma_eng = nc.sync if ch == 0 else nc.gpsimd
                dma_eng.dma_start(
                    out=out_ap[row0:row0 + 128, ch * 512:(ch + 1) * 512],
                    in_=pout,
                )
                yield

    def emit_tail_proj():
        """Last chunk's projection: all 8 groups' p=0..2 accumulations are
        emitted FIRST (they only need stacks 0..2, so they overlap the final
        norm chain on the PE), then the p=3 matmuls, then the evacuations.
        Eight PSUM banks: 2 ps + 2 y + halves of 2 st tiles."""
        qc = NQC - 1
        stacks = stacks_store[qc]
        groups = []
        st_tiles = []
        for tq in range(4):
            for ch in range(2):
                gi = tq * 2 + ch
                if gi < 2:
                    ps = ps_pool.tile([128, 512], F32, tag="ps",
                                      name=f"tp_ps_{gi}")
                elif gi < 4:
                    ps = y_pool.tile([128, 512], F32, tag="y",
                                     name=f"tp_y_{gi}")
                else:
                    if gi % 2 == 0:
                        st_tiles.append(st_pool.tile(
                            [128, 1024], F32, tag="st", name=f"tp_st_{gi}"))
                    half = st_tiles[-1]
                    ps = half[:, 0:512] if gi % 2 == 0 else half[:, 512:1024]
                for p in range(3):
                    nc.tensor.matmul(
                        ps,
                        stacks[p][:, tq * 128:(tq + 1) * 128],
                        wp_sb[:, p, ch * 512:(ch + 1) * 512],
                        start=(p == 0),
                        stop=False,
                    )
                groups.append((ps, tq, ch))
        for ps, tq, ch in groups:
            nc.tensor.matmul(
                ps,
                stacks[3][:, tq * 128:(tq + 1) * 128],
                wp_sb[:, 3, ch * 512:(ch + 1) * 512],
                start=False,
                stop=True,
            )
        for gi, (ps, tq, ch) in enumerate(groups):
            row0 = qc * 512 + tq * 128
            pout = pout_pool.tile([128, 512], F32, tag="pout",
                                  name=f"tp_po_{tq}_{ch}")
            if gi % 2 == 0:
                nc.vector.tensor_copy(pout, ps)
            else:
                nc.scalar.copy(pout, ps)
            dma_eng = nc.sync if ch == 0 else nc.gpsimd
            dma_eng.dma_start(
                out=out_ap[row0:row0 + 128, ch * 512:(ch + 1) * 512],
                in_=pout,
            )

    gen0 = gen_chunk(0)
    next(gen0)          # x(0) loads onto the scalar queue first
    load_weights()      # then the weight blocks, split across both queues
    for _ in gen0:
        pass

    fillers = []

    def pull(n):
        for _ in range(n):
            while fillers:
                try:
                    next(fillers[0])
                    break
                except StopIteration:
                    fillers.pop(0)

    for qc in range(NQC):
        stacks_store[qc] = [None] * 4
        if qc > 0:
            fillers.append(gen_norm(qc - 1, 2))
            fillers.append(gen_norm(qc - 1, 3))
        if qc + 1 < NQC:
            fillers.append(gen_chunk(qc + 1))
        if qc > 0:
            fillers.append(gen_proj(qc - 1))

        # ---- attention for this query chunk, per head pair ----
        nki = 4 * qc + 4
        qts = qts_store[qc]
        for p in range(HPC // 2):
            qt = qts[p]
            yA = y_pool.tile([128, 512], F32, tag="y", name=f"yA_{qc}_{p}")
            yB = y_pool.tile([128, 512], F32, tag="y", name=f"yB_{qc}_{p}")
            for ki in range(nki):
                r = ki - 4 * qc  # >= 0 on the block diagonal
                n0 = 128 * r if r > 0 else 0
                st = st_pool.tile([128, 1024], F32, tag="st",
                                  name=f"st_{qc}_{p}_{ki}")
                st3 = st.rearrange("p (h q) -> p h q", h=2)
                kslice = slice(ki * 128, (ki + 1) * 128)
                nc.tensor.matmul(
                    st3[:, 0, n0:512], kt[0:64, p, kslice], qt[0:64, n0:512],
                    start=True, stop=True,
                )
                nc.tensor.matmul(
                    st3[:, 1, n0:512], kt[64:128, p, kslice], qt[64:128, n0:512],
                    start=True, stop=True,
                )
                se = se_pool.tile([128, 1024], BF16, tag="se",
                                  name=f"se_{qc}_{p}_{ki}")
                se3 = se.rearrange("p (h q) -> p h q", h=2)
                nc.scalar.activation(
                    se3[:, :, n0:512], st3[:, :, n0:512], Exp,
                    bias=adj_sb[:, ki:ki + 1], scale=SCALE,
                )
                if r >= 0:
                    nc.vector.tensor_mul(
                        se3[:, :, n0:n0 + 128], se3[:, :, n0:n0 + 128], tri_b
                    )
                nc.tensor.matmul(
                    yA[0:HD + 1, n0:512], v2[:, ki, 2 * p, :], se3[:, 0, n0:512],
                    start=(ki == 0), stop=(ki == nki - 1), skip_group_check=True,
                )
                nc.tensor.matmul(
                    yB[0:HD + 1, n0:512], v2[:, ki, 2 * p + 1, :], se3[:, 1, n0:512],
                    start=(ki == 0), stop=(ki == nki - 1), skip_group_check=True,
                )
                # cadence: spread the filler items over the chunk's ki slots
                if qc == 0:
                    pull(2)
                elif qc == 1:
                    pull(2 if ki % 2 == 0 else 1)
                elif qc == 2:
                    pull(1)
                else:
                    # reserve fillers for the ACT-bound back half of the
                    # last chunk, where the PE otherwise runs dry
                    pull(1 if (ki >= 8 and ki % 2 == 0) else 0)

            # evacuate y^T (+ row HD denominators); everything else is
            # deferred to gen_norm pulled >= 2 pairs later
            tmpA = tmp_pool.tile([65, 512], F32, tag="tmp", name=f"tmpA_{qc}_{p}")
            nc.vector.tensor_copy(tmpA[0:HD + 1, :], yA[0:HD + 1, :])
            tmpB = tmp_pool.tile([65, 512], F32, tag="tmp", name=f"tmpB_{qc}_{p}")
            nc.vector.tensor_copy(tmpB[0:HD + 1, :], yB[0:HD + 1, :])
            tmps_store[(qc, p)] = (tmpA, tmpB)
            if p < 2 or qc == NQC - 1:
                fillers.append(gen_norm(qc, p))
            pull(2)

        pull(1000)

    # ---- tail: last chunk's projection ----
    emit_tail_proj()


def make_nc():
    nc = bacc.Bacc("TRN2", target_bir_lowering=False, debug=False,
                   num_devices=NCORES)
    xT = nc.dram_tensor("xT", [C, T], BF16, kind="ExternalInput")
    wqkv = nc.dram_tensor("wqkv", [C, 3 * FPC], BF16, kind="ExternalInput")
    wproj = nc.dram_tensor("wproj", [FPC, C], BF16, kind="ExternalInput")
    adj = nc.dram_tensor("adj", [128, NKT], F32, kind="ExternalInput")
    out = nc.dram_tensor("out", [T, C], F32, kind="ExternalOutput")
    tri_np = np.triu(np.ones((128, 128), dtype=np.float32))
    tri_dram = nc.inline_tensor(tri_np, name="tri_const")
    with ExitStack() as ctx:
        tc = ctx.enter_context(tile.TileContext(nc))
        tc.ctx = ctx
        build(tc, out[:, :], xT[:, :], wqkv[:, :], wproj[:, :], adj[:],
              tri_dram)
    nc.compile()
    return nc


def shard_inputs(x, prev_probs, W_attn, W_proj):
    import ml_dtypes

    in_maps = []
    for core in range(NCORES):
        b, g = divmod(core, 2)
        xT = np.ascontiguousarray(x[b].T)
        wq = W_attn[:, g * FPC:(g + 1) * FPC]
        wk = W_attn[:, C + g * FPC:C + (g + 1) * FPC]
        wv = W_attn[:, 2 * C + g * FPC:2 * C + (g + 1) * FPC]
        wqkv = np.ascontiguousarray(np.concatenate([wq, wk, wv], axis=1))
        wproj = np.ascontiguousarray(W_proj[g * FPC:(g + 1) * FPC, :])
        adj = (-np.float32(EPS_BIAS)
               * np.log(prev_probs[b] + np.float32(1e-10))).astype(np.float32)
        # pre-shuffle to adj_sb layout: [key-within-tile (partition), tile]
        adj = np.ascontiguousarray(adj.reshape(NKT, 128).T)
        in_maps.append(
            {
                "xT": xT.astype(ml_dtypes.bfloat16),
                "wqkv": wqkv.astype(ml_dtypes.bfloat16),
                "wproj": wproj.astype(ml_dtypes.bfloat16),
                "adj": adj,
            }
        )
    return in_maps


_CACHED_NC = None


def kernel(x, prev_probs, W_attn, W_proj, trace=False, tmpdir=None):
    global _CACHED_NC
    from concourse.bass_utils import run_bass_kernel_spmd

    x = np.asarray(x, dtype=np.float32)
    prev_probs = np.asarray(prev_probs, dtype=np.float32)
    W_attn = np.asarray(W_attn, dtype=np.float32)
    W_proj = np.asarray(W_proj, dtype=np.float32)

    if _CACHED_NC is None:
        _CACHED_NC = make_nc()
    nc = _CACHED_NC

    in_maps = shard_inputs(x, prev_probs, W_attn, W_proj)
    res = run_bass_kernel_spmd(
        nc, in_maps, core_ids=list(range(NCORES)), trace=trace, tmpdir=tmpdir
    )
    parts = [r["out"] for r in res.results]
    out = np.empty((B, T, C), dtype=np.float32)
    for b in range(B):
        out[b] = parts[2 * b] + parts[2 * b + 1]
    kernel.last_results = res
    return out


# revision 64
# speedup vs baseline: 1.0509x; 1.0237x over previous
"""Trainium2 Bass kernel for CausalSelfAttention (B=4, T=2048, C=1024, H=16)
with additive prev-prob key bias.

Sharding: 8 cores = data-parallel over B (4) x tensor-parallel over head
halves (2).  Each core computes qkv for its 8 heads, causal attention, and a
partial output projection (row-parallel W_proj); host sums the two partials
per batch at unshard time.

Per-core device algorithm (HAM-warm bf16 schedule):
  - All matmuls run bf16 x bf16 with fp32 PSUM accumulation (rel err
    ~4e-3 vs the 2e-2 gate).  K^T and Q^T are kept feature-major with
    head pairs stacked in the 128 partitions, so QK^T runs as two
    row-tiled (K=64) concurrent matmuls per issue slot.
  - Scores are computed transposed (keys on partitions).  The additive
    prev-prob bias folds into the exp as a per-partition bias AP
    (exp(qk/8 + adj[k])); the softmax denominator comes from a constant
    ones column appended to V (M=65 PV matmuls).
  - Causality: block-trimmed matmul widths + one fused 128x(2x128)
    triangular mask multiply per diagonal block.
  - Normalization is fully deferred: each head pair only evacuates y^T;
    2+ pairs later a filler chain bridges the denominator rows to
    partition 0 (tiny DMA), reciprocal_approx_fast (DVE, zero-wait),
    partition_broadcast (gpsimd ucode — the ONLY custom gpsimd op, so
    its library stays resident), and base-0 stack muls (DVE).  No engine
    FIFO ever blocks on DMA latency, so HAM stays at K=8/8.
  - Startup: warmup matmuls un-throttle the PE during the initial DMAs;
    weight blocks are split across both HWDGE queues (sync: Wq+Wv+Wproj,
    scalar: x(0)+Wk) so every block lands just before its first use; the
    adj bias is pre-shuffled on the host to avoid a gather DMA.
  - Queue topology: x loads ride gpsimd SWDGE (chunks 1+), projection
    output DMAs alternate sync/gpsimd, evacuations stay on DVE so the
    scalar engine runs pure exp (its ~165us is the #2 engine floor).
"""

import math
from contextlib import ExitStack

import numpy as np

import concourse.bass as bass
import concourse.tile as tile
from concourse import bacc, mybir

F32 = mybir.dt.float32
F32R = mybir.dt.float32r
BF16 = mybir.dt.bfloat16

USE_F32R = True
MMD = F32R if USE_F32R else F32

B, T, C, H = 4, 2048, 1024, 16
HD = C // H          # 64
NCORES = 8
HPC = H // 2         # 8 heads per core
FPC = HPC * HD       # 512 features per core
NKT = T // 128       # 16 key tiles
NQC = T // 512       # 4 query chunks (also the x t-chunks)
NCT = C // 128       # 8 contraction tiles
EPS_BIAS = 0.1
SCALE = 1.0 / math.sqrt(HD)


def build(tc, out_ap, xT, wqkv, wproj, adj, tri_dram):
    """Emit the per-core kernel into TileContext tc.

    out_ap : (T, C)    partial projection output (needs pair-sum on host)
    xT     : (C, T)    x[b] transposed
    wqkv   : (C, 3*FPC) [Wq_g | Wk_g | Wv_g] columns for this head group
    wproj  : (FPC, C)  W_proj rows for this head group
    adj    : (T,)      -EPS_BIAS * log(prev_probs[b] + 1e-10)
    tri_dram: (128,128) upper-triangular ones (tri[k,q] = 1 iff k <= q)
    """
    nc = tc.nc
    ctx = tc.ctx
    Exp = mybir.ActivationFunctionType.Exp

    const = ctx.enter_context(tc.tile_pool(name="const", bufs=1))
    xs_pool = ctx.enter_context(tc.tile_pool(name="xs", bufs=2))
    qt_pool = ctx.enter_context(tc.tile_pool(name="qt", bufs=10))
    se_pool = ctx.enter_context(tc.tile_pool(name="se", bufs=6))
    tmp_pool = ctx.enter_context(tc.tile_pool(name="tmp", bufs=8))
    stkb_pool = ctx.enter_context(tc.tile_pool(name="stkb", bufs=3))
    scale_pool = ctx.enter_context(tc.tile_pool(name="scale", bufs=4))
    stack_pool = ctx.enter_context(tc.tile_pool(name="stack", bufs=8))
    pout_pool = ctx.enter_context(tc.tile_pool(name="pout", bufs=4))

    ps_pool = ctx.enter_context(tc.tile_pool(name="ps", bufs=2, space="PSUM"))
    st_pool = ctx.enter_context(tc.tile_pool(name="st", bufs=2, space="PSUM"))
    y_pool = ctx.enter_context(tc.tile_pool(name="y", bufs=2, space="PSUM"))

    # ---- PE warmup: drive HAM to K=8/8 while the initial DMAs stream ----
    warm_w = const.tile([128, 128], F32, name="warm_w")
    nc.vector.memset(warm_w, 0.25)
    wps = ps_pool.tile([128, 512], F32, tag="ps", name="warm_ps")
    for i in range(40):
        nc.tensor.matmul(wps[:, 0:128], warm_w, warm_w, start=True, stop=True)

    # ---- persistent buffers (loads emitted after chunk-0 x loads) ----
    wq_sb = const.tile([128, NCT, 3 * FPC], BF16, name="wq_sb")     # 24KB/p
    wp_sb = const.tile([128, FPC // 128, C], BF16, name="wp_sb")    # 8KB/p
    kt = const.tile([128, HPC // 2, T], BF16, name="kt")            # 16KB/p
    # constant ones column -> PV accumulates the softmax denominator
    v2 = const.tile([128, NKT, HPC, HD + 1], BF16, name="v2")       # 16.6KB/p
    nc.vector.memset(v2[:, :, :, HD:HD + 1], 1.0)
    adj_sb = const.tile([128, NKT], F32, name="adj_sb")
    dn = const.tile([1, 1024], F32, name="dn")  # den rows bridged to part 0
    tri = const.tile([128, 128], F32, name="tri")
    tri_bf = const.tile([128, 128], BF16, name="tri_bf")
    tri_b = tri_bf.unsqueeze(1).to_broadcast([128, 2, 128])

    def load_weights():
        """Queue plan: sync = Wq + Wv + Wproj, scalar = x(0) (already
        queued) + Wk + adj + tri.  One DMA per weight block — small DMAs
        are dispatch/credit-limited (~1.4us each), not bandwidth-limited."""
        nc.scalar.dma_start(out=adj_sb, in_=adj[:, :])
        nc.scalar.dma_start(out=tri, in_=tri_dram[:, :])
        wqkv3 = wqkv.rearrange("(c p) f -> p c f", p=128)
        for blk, eng in ((0, nc.sync), (2, nc.sync), (1, nc.scalar)):
            eng.dma_start(
                out=wq_sb[:, :, blk * FPC:(blk + 1) * FPC],
                in_=wqkv3[:, :, blk * FPC:(blk + 1) * FPC],
            )
        nc.sync.dma_start(
            out=wp_sb, in_=wproj.rearrange("(i p) c -> p i c", p=128)
        )

    qts_store = {}
    stacks_store = {}
    tmps_store = {}

    def gen_chunk(qc):
        """Emit one t-chunk's pre-attention work as resumable items:
        x loads, JIT Q^T, K^T tiles, V tiles.  x loads alternate between the
        two HWDGE queues, and spacer yields separate them from the first
        matmul group so the PE FIFO never waits on an in-flight load."""
        xs_eng = nc.scalar if qc <= 1 else nc.gpsimd
        xsc = xs_pool.tile([128, NCT, 512], BF16, tag="xs", name=f"xs_{qc}")
        xs_eng.dma_start(
            out=xsc,
            in_=xT[:, qc * 512:(qc + 1) * 512].rearrange(
                "(c p) q -> p c q", p=128),
        )
        xs_tiles = [xsc[:, c, :] for c in range(NCT)]
        for _ in range(7 if qc > 0 else 1):
            yield
        qts = []
        for p in range(HPC // 2):
            ps = ps_pool.tile([128, 512], F32, tag="ps", name=f"qps_{qc}_{p}")
            for c in range(NCT):
                nc.tensor.matmul(
                    ps,
                    wq_sb[:, c, p * 128:(p + 1) * 128],
                    xs_tiles[c],
                    start=(c == 0),
                    stop=(c == NCT - 1),
                )
            qt = qt_pool.tile([128, 512], BF16, tag="qt", name=f"qt_{qc}_{p}")
            nc.vector.tensor_copy(qt, ps)
            qts.append(qt)
            yield
        qts_store[qc] = qts
        for p in range(HPC // 2):
            ps = ps_pool.tile([128, 512], F32, tag="ps", name=f"kps_{qc}_{p}")
            for c in range(NCT):
                nc.tensor.matmul(
                    ps,
                    wq_sb[:, c, FPC + p * 128:FPC + (p + 1) * 128],
                    xs_tiles[c],
                    start=(c == 0),
                    stop=(c == NCT - 1),
                )
            nc.vector.tensor_copy(kt[:, p, qc * 512:(qc + 1) * 512], ps)
            yield
        for j in range(4):
            kt_i = qc * 4 + j
            ps = ps_pool.tile([128, 512], F32, tag="ps", name=f"vps_{qc}_{j}")
            for c in range(NCT):
                nc.tensor.matmul(
                    ps,
                    xs_tiles[c][:, j * 128:(j + 1) * 128],
                    wq_sb[:, c, 2 * FPC:3 * FPC],
                    start=(c == 0),
                    stop=(c == NCT - 1),
                )
            nc.vector.tensor_copy(
                v2[:, kt_i, :, 0:HD],
                ps.rearrange("p (h d) -> p h d", h=HPC),
            )
            yield

    def gen_norm(qc, p):
        """Deferred denominator chain for head pair (qc, p): DMA-bridge the
        den rows (partition 64) to partition 0, reciprocal there (DVE),
        partition-broadcast to rows 0..63 (gpsimd, SBUF-to-SBUF), base-0
        stack muls (gpsimd).  Pulled 2+ pairs after the pair finished, so
        nothing ever stalls."""
        tmpA, tmpB = tmps_store[(qc, p)]
        nc.sync.dma_start(out=dn[0:1, 0:512], in_=tmpA[HD:HD + 1, :])
        nc.sync.dma_start(out=dn[0:1, 512:1024], in_=tmpB[HD:HD + 1, :])
        yield
        sc = scale_pool.tile([64, 1024], F32, tag="scale", name=f"sc_{qc}_{p}")
        nc.vector.reciprocal_approx_fast(sc[0:1, :], dn[0:1, :])
        yield
        nc.gpsimd.partition_broadcast(sc[0:64, 0:512], sc[0:1, 0:512],
                                      channels=64)
        nc.gpsimd.partition_broadcast(sc[0:64, 512:1024], sc[0:1, 512:1024],
                                      channels=64)
        yield
        stack = stack_pool.tile([128, 512], BF16, tag="stack", name=f"stk_{qc}_{p}")
        nc.vector.tensor_mul(stack[0:64, :], tmpA[0:64, :], sc[0:64, 0:512])
        yield
        stkB = stkb_pool.tile([64, 512], BF16, tag="stkB", name=f"skB_{qc}_{p}")
        nc.vector.tensor_mul(stkB[0:64, :], tmpB[0:64, :], sc[0:64, 512:1024])
        nc.sync.dma_start(out=stack[64:128, :], in_=stkB[0:64, :])
        stacks_store[qc][p] = stack
        yield

    def gen_proj(qc):
        stacks = stacks_store[qc]
        for tq in range(4):
            row0 = qc * 512 + tq * 128
            for ch in range(2):
                ps = ps_pool.tile([128, 512], F32, tag="ps",
                                  name=f"pps_{qc}_{tq}_{ch}")
                for p in range(HPC // 2):
                    nc.tensor.matmul(
                        ps,
                        stacks[p][:, tq * 128:(tq + 1) * 128],
                        wp_sb[:, p, ch * 512:(ch + 1) * 512],
                        start=(p == 0),
                        stop=(p == HPC // 2 - 1),
                    )
                pout = pout_pool.tile([128, 512], F32, tag="pout",
                                      name=f"po_{qc}_{tq}_{ch}")
                nc.vector.tensor_copy(pout, ps)
                # alternate DMA queue to break the pout-copy <-> out-DMA
                # recycle round-robin
                dma_eng = nc.sync if ch == 0 else nc.gpsimd
                dma_eng.dma_start(
                    out=out_ap[row0:row0 + 128, ch * 512:(ch + 1) * 512],
                    in_=pout,
                )
                yield

    def emit_tail_proj():
        """Last chunk's projection: all 8 groups' p=0..2 accumulations are
        emitted FIRST (they only need stacks 0..2, so they overlap the final
        norm chain on the PE), then the p=3 matmuls, then the evacuations.
        Eight PSUM banks: 2 ps + 2 y + halves of 2 st tiles."""
        qc = NQC - 1
        stacks = stacks_store[qc]
        groups = []
        st_tiles = []
        for tq in range(4):
            for ch in range(2):
                gi = tq * 2 + ch
                if gi < 2:
                    ps = ps_pool.tile([128, 512], F32, tag="ps",
                                      name=f"tp_ps_{gi}")
                elif gi < 4:
                    ps = y_pool.tile([128, 512], F32, tag="y",
                                     name=f"tp_y_{gi}")
                else:
                    if gi % 2 == 0:
                        st_tiles.append(st_pool.tile(
                            [128, 1024], F32, tag="st", name=f"tp_st_{gi}"))
                    half = st_tiles[-1]
                    ps = half[:, 0:512] if gi % 2 == 0 else half[:, 512:1024]
                for p in range(3):
                    nc.tensor.matmul(
                        ps,
                        stacks[p][:, tq * 128:(tq + 1) * 128],
                        wp_sb[:, p, ch * 512:(ch + 1) * 512],
                        start=(p == 0),
                        stop=False,
                    )
                groups.append((ps, tq, ch))
        for ps, tq, ch in groups:
            nc.tensor.matmul(
                ps,
                stacks[3][:, tq * 128:(tq + 1) * 128],
                wp_sb[:, 3, ch * 512:(ch + 1) * 512],
                start=False,
                stop=True,
            )
        for gi, (ps, tq, ch) in enumerate(groups):
            row0 = qc * 512 + tq * 128
            pout = pout_pool.tile([128, 512], F32, tag="pout",
                                  name=f"tp_po_{tq}_{ch}")
            if gi % 2 == 0:
                nc.vector.tensor_copy(pout, ps)
            else:
                nc.scalar.copy(pout, ps)
            dma_eng = nc.sync if ch == 0 else nc.gpsimd
            dma_eng.dma_start(
                out=out_ap[row0:row0 + 128, ch * 512:(ch + 1) * 512],
                in_=pout,
            )

    gen0 = gen_chunk(0)
    next(gen0)          # x(0) loads onto the scalar queue first
    load_weights()      # then the weight blocks, split across both queues
    for _ in gen0:
        pass
    # cast emitted after chunk-0 prep so it can't head-of-line block the
    # early qt/kt evacuations in the DVE FIFO
    nc.vector.tensor_copy(tri_bf, tri)

    fillers = []

    def pull(n):
        for _ in range(n):
            while fillers:
                try:
                    next(fillers[0])
                    break
                except StopIteration:
                    fillers.pop(0)

    for qc in range(NQC):
        stacks_store[qc] = [None] * 4
        if qc > 0:
            fillers.append(gen_norm(qc - 1, 2))
            fillers.append(gen_norm(qc - 1, 3))
        if qc + 1 < NQC:
            fillers.append(gen_chunk(qc + 1))
        if qc > 0:
            fillers.append(gen_proj(qc - 1))

        # ---- attention for this query chunk, per head pair ----
        nki = 4 * qc + 4
        qts = qts_store[qc]
        for p in range(HPC // 2):
            qt = qts[p]
            yA = y_pool.tile([128, 512], F32, tag="y", name=f"yA_{qc}_{p}")
            yB = y_pool.tile([128, 512], F32, tag="y", name=f"yB_{qc}_{p}")
            for ki in range(nki):
                r = ki - 4 * qc  # >= 0 on the block diagonal
                n0 = 128 * r if r > 0 else 0
                st = st_pool.tile([128, 1024], F32, tag="st",
                                  name=f"st_{qc}_{p}_{ki}")
                st3 = st.rearrange("p (h q) -> p h q", h=2)
                kslice = slice(ki * 128, (ki + 1) * 128)
                nc.tensor.matmul(
                    st3[:, 0, n0:512], kt[0:64, p, kslice], qt[0:64, n0:512],
                    start=True, stop=True,
                )
                nc.tensor.matmul(
                    st3[:, 1, n0:512], kt[64:128, p, kslice], qt[64:128, n0:512],
                    start=True, stop=True,
                )
                se = se_pool.tile([128, 1024], BF16, tag="se",
                                  name=f"se_{qc}_{p}_{ki}")
                se3 = se.rearrange("p (h q) -> p h q", h=2)
                nc.scalar.activation(
                    se3[:, :, n0:512], st3[:, :, n0:512], Exp,
                    bias=adj_sb[:, ki:ki + 1], scale=SCALE,
                )
                if r >= 0:
                    nc.vector.tensor_mul(
                        se3[:, :, n0:n0 + 128], se3[:, :, n0:n0 + 128], tri_b
                    )
                nc.tensor.matmul(
                    yA[0:HD + 1, n0:512], v2[:, ki, 2 * p, :], se3[:, 0, n0:512],
                    start=(ki == 0), stop=(ki == nki - 1), skip_group_check=True,
                )
                nc.tensor.matmul(
                    yB[0:HD + 1, n0:512], v2[:, ki, 2 * p + 1, :], se3[:, 1, n0:512],
                    start=(ki == 0), stop=(ki == nki - 1), skip_group_check=True,
                )
                # cadence: spread the filler items over the chunk's ki slots
                if qc == 0:
                    pull(2)
                elif qc == 1:
                    pull(2 if ki % 2 == 0 else 1)
                elif qc == 2:
                    pull(1)
                else:
                    # reserve fillers for the ACT-bound back half of the
                    # last chunk, where the PE otherwise runs dry
                    pull(1 if (ki >= 8 and ki % 2 == 0) else 0)

            # evacuate y^T (+ row HD denominators); everything else is
            # deferred to gen_norm pulled >= 2 pairs later
            tmpA = tmp_pool.tile([65, 512], F32, tag="tmp", name=f"tmpA_{qc}_{p}")
            nc.vector.tensor_copy(tmpA[0:HD + 1, :], yA[0:HD + 1, :])
            tmpB = tmp_pool.tile([65, 512], F32, tag="tmp", name=f"tmpB_{qc}_{p}")
            nc.vector.tensor_copy(tmpB[0:HD + 1, :], yB[0:HD + 1, :])
            tmps_store[(qc, p)] = (tmpA, tmpB)
            if p < 2 or qc == NQC - 1:
                fillers.append(gen_norm(qc, p))
            pull(2)

        pull(1000)

    # ---- tail: last chunk's projection ----
    emit_tail_proj()


def make_nc():
    nc = bacc.Bacc("TRN2", target_bir_lowering=False, debug=False,
                   num_devices=NCORES)
    xT = nc.dram_tensor("xT", [C, T], BF16, kind="ExternalInput")
    wqkv = nc.dram_tensor("wqkv", [C, 3 * FPC], BF16, kind="ExternalInput")
    wproj = nc.dram_tensor("wproj", [FPC, C], BF16, kind="ExternalInput")
    adj = nc.dram_tensor("adj", [128, NKT], F32, kind="ExternalInput")
    out = nc.dram_tensor("out", [T, C], F32, kind="ExternalOutput")
    tri_np = np.triu(np.ones((128, 128), dtype=np.float32))
    tri_dram = nc.inline_tensor(tri_np, name="tri_const")
    with ExitStack() as ctx:
        tc = ctx.enter_context(tile.TileContext(nc))
        tc.ctx = ctx
        build(tc, out[:, :], xT[:, :], wqkv[:, :], wproj[:, :], adj[:],
              tri_dram)
    nc.compile()
    return nc


def shard_inputs(x, prev_probs, W_attn, W_proj):
    import ml_dtypes

    in_maps = []
    for core in range(NCORES):
        b, g = divmod(core, 2)
        xT = np.ascontiguousarray(x[b].T)
        wq = W_attn[:, g * FPC:(g + 1) * FPC]
        wk = W_attn[:, C + g * FPC:C + (g + 1) * FPC]
        wv = W_attn[:, 2 * C + g * FPC:2 * C + (g + 1) * FPC]
        wqkv = np.ascontiguousarray(np.concatenate([wq, wk, wv], axis=1))
        wproj = np.ascontiguousarray(W_proj[g * FPC:(g + 1) * FPC, :])
        adj = (-np.float32(EPS_BIAS)
               * np.log(prev_probs[b] + np.float32(1e-10))).astype(np.float32)
        # pre-shuffle to adj_sb layout: [key-within-tile (partition), tile]
        adj = np.ascontiguousarray(adj.reshape(NKT, 128).T)
        in_maps.append(
            {
                "xT": xT.astype(ml_dtypes.bfloat16),
                "wqkv": wqkv.astype(ml_dtypes.bfloat16),
                "wproj": wproj.astype(ml_dtypes.bfloat16),
                "adj": adj,
            }
        )
    return in_maps


_CACHED_NC = None


def kernel(x, prev_probs, W_attn, W_proj, trace=False, tmpdir=None):
    global _CACHED_NC
    from concourse.bass_utils import run_bass_kernel_spmd

    x = np.asarray(x, dtype=np.float32)
    prev_probs = np.asarray(prev_probs, dtype=np.float32)
    W_attn = np.asarray(W_attn, dtype=np.float32)
    W_proj = np.asarray(W_proj, dtype=np.float32)

    if _CACHED_NC is None:
        _CACHED_NC = make_nc()
    nc = _CACHED_NC

    in_maps = shard_inputs(x, prev_probs, W_attn, W_proj)
    res = run_bass_kernel_spmd(
        nc, in_maps, core_ids=list(range(NCORES)), trace=trace, tmpdir=tmpdir
    )
    parts = [r["out"] for r in res.results]
    out = np.empty((B, T, C), dtype=np.float32)
    for b in range(B):
        out[b] = parts[2 * b] + parts[2 * b + 1]
    kernel.last_results = res
    return out


# revision 65
# speedup vs baseline: 1.0557x; 1.0045x over previous
"""Trainium2 Bass kernel for CausalSelfAttention (B=4, T=2048, C=1024, H=16)
with additive prev-prob key bias.

Sharding: 8 cores = data-parallel over B (4) x tensor-parallel over head
halves (2).  Each core computes qkv for its 8 heads, causal attention, and a
partial output projection (row-parallel W_proj); host sums the two partials
per batch at unshard time.

Per-core device algorithm (HAM-warm bf16 schedule):
  - All matmuls run bf16 x bf16 with fp32 PSUM accumulation (rel err
    ~4e-3 vs the 2e-2 gate).  K^T and Q^T are kept feature-major with
    head pairs stacked in the 128 partitions, so QK^T runs as two
    row-tiled (K=64) concurrent matmuls per issue slot.
  - Scores are computed transposed (keys on partitions).  The additive
    prev-prob bias folds into the exp as a per-partition bias AP
    (exp(qk/8 + adj[k])); the softmax denominator comes from a constant
    ones column appended to V (M=65 PV matmuls).
  - Causality: block-trimmed matmul widths + one fused 128x(2x128)
    triangular mask multiply per diagonal block.
  - Normalization is fully deferred: each head pair only evacuates y^T;
    2+ pairs later a filler chain bridges the denominator rows to
    partition 0 (tiny DMA), reciprocal_approx_fast (DVE, zero-wait),
    partition_broadcast (gpsimd ucode — the ONLY custom gpsimd op, so
    its library stays resident), and base-0 stack muls (DVE).  No engine
    FIFO ever blocks on DMA latency, so HAM stays at K=8/8.
  - Startup: warmup matmuls un-throttle the PE during the initial DMAs;
    weight blocks are split across both HWDGE queues (sync: Wq+Wv+Wproj,
    scalar: x(0)+Wk) so every block lands just before its first use; the
    adj bias is pre-shuffled on the host to avoid a gather DMA.
  - Queue topology: x loads ride gpsimd SWDGE (chunks 1+), projection
    output DMAs alternate sync/gpsimd, evacuations stay on DVE so the
    scalar engine runs pure exp (its ~165us is the #2 engine floor).
"""

import math
from contextlib import ExitStack

import numpy as np

import concourse.bass as bass
import concourse.tile as tile
from concourse import bacc, mybir

F32 = mybir.dt.float32
F32R = mybir.dt.float32r
BF16 = mybir.dt.bfloat16

USE_F32R = True
MMD = F32R if USE_F32R else F32

B, T, C, H = 4, 2048, 1024, 16
HD = C // H          # 64
NCORES = 8
HPC = H // 2         # 8 heads per core
FPC = HPC * HD       # 512 features per core
NKT = T // 128       # 16 key tiles
NQC = T // 512       # 4 query chunks (also the x t-chunks)
NCT = C // 128       # 8 contraction tiles
EPS_BIAS = 0.1
SCALE = 1.0 / math.sqrt(HD)


def build(tc, out_ap, xT, wqkv, wproj, adj, tri_dram):
    """Emit the per-core kernel into TileContext tc.

    out_ap : (T, C)    partial projection output (needs pair-sum on host)
    xT     : (C, T)    x[b] transposed
    wqkv   : (C, 3*FPC) [Wq_g | Wk_g | Wv_g] columns for this head group
    wproj  : (FPC, C)  W_proj rows for this head group
    adj    : (T,)      -EPS_BIAS * log(prev_probs[b] + 1e-10)
    tri_dram: (128,128) upper-triangular ones (tri[k,q] = 1 iff k <= q)
    """
    nc = tc.nc
    ctx = tc.ctx
    Exp = mybir.ActivationFunctionType.Exp

    const = ctx.enter_context(tc.tile_pool(name="const", bufs=1))
    xs_pool = ctx.enter_context(tc.tile_pool(name="xs", bufs=2))
    qt_pool = ctx.enter_context(tc.tile_pool(name="qt", bufs=10))
    se_pool = ctx.enter_context(tc.tile_pool(name="se", bufs=6))
    tmp_pool = ctx.enter_context(tc.tile_pool(name="tmp", bufs=8))
    stkb_pool = ctx.enter_context(tc.tile_pool(name="stkb", bufs=3))
    scale_pool = ctx.enter_context(tc.tile_pool(name="scale", bufs=4))
    stack_pool = ctx.enter_context(tc.tile_pool(name="stack", bufs=8))
    pout_pool = ctx.enter_context(tc.tile_pool(name="pout", bufs=4))

    ps_pool = ctx.enter_context(tc.tile_pool(name="ps", bufs=2, space="PSUM"))
    st_pool = ctx.enter_context(tc.tile_pool(name="st", bufs=2, space="PSUM"))
    y_pool = ctx.enter_context(tc.tile_pool(name="y", bufs=2, space="PSUM"))

    # ---- PE warmup: drive HAM to K=8/8 while the initial DMAs stream ----
    warm_w = const.tile([128, 128], F32, name="warm_w")
    nc.vector.memset(warm_w, 0.25)
    wps = ps_pool.tile([128, 512], F32, tag="ps", name="warm_ps")
    for i in range(40):
        nc.tensor.matmul(wps[:, 0:128], warm_w, warm_w, start=True, stop=True)

    # ---- persistent buffers (loads emitted after chunk-0 x loads) ----
    wq_sb = const.tile([128, NCT, 3 * FPC], BF16, name="wq_sb")     # 24KB/p
    wp_sb = const.tile([128, FPC // 128, C], BF16, name="wp_sb")    # 8KB/p
    kt = const.tile([128, HPC // 2, T], BF16, name="kt")            # 16KB/p
    # constant ones column -> PV accumulates the softmax denominator
    v2 = const.tile([128, NKT, HPC, HD + 1], BF16, name="v2")       # 16.6KB/p
    nc.vector.memset(v2[:, :, :, HD:HD + 1], 1.0)
    adj_sb = const.tile([128, NKT], F32, name="adj_sb")
    dn = const.tile([1, 1024], F32, name="dn")  # den rows bridged to part 0
    tri = const.tile([128, 128], F32, name="tri")
    tri_bf = const.tile([128, 128], BF16, name="tri_bf")
    tri_b = tri_bf.unsqueeze(1).to_broadcast([128, 2, 128])

    def load_weights():
        """Queue plan: sync = Wq + Wv + Wproj, scalar = x(0) (already
        queued) + Wk + adj + tri.  One DMA per weight block — small DMAs
        are dispatch/credit-limited (~1.4us each), not bandwidth-limited."""
        nc.scalar.dma_start(out=adj_sb, in_=adj[:, :])
        nc.scalar.dma_start(out=tri, in_=tri_dram[:, :])
        wqkv3 = wqkv.rearrange("(c p) f -> p c f", p=128)
        for blk, eng in ((0, nc.sync), (2, nc.sync), (1, nc.scalar)):
            eng.dma_start(
                out=wq_sb[:, :, blk * FPC:(blk + 1) * FPC],
                in_=wqkv3[:, :, blk * FPC:(blk + 1) * FPC],
            )
        nc.sync.dma_start(
            out=wp_sb, in_=wproj.rearrange("(i p) c -> p i c", p=128)
        )

    qts_store = {}
    stacks_store = {}
    tmps_store = {}

    def gen_chunk(qc):
        """Emit one t-chunk's pre-attention work as resumable items:
        x loads, JIT Q^T, K^T tiles, V tiles.  x loads alternate between the
        two HWDGE queues, and spacer yields separate them from the first
        matmul group so the PE FIFO never waits on an in-flight load."""
        xs_eng = nc.scalar if qc <= 1 else nc.gpsimd
        xsc = xs_pool.tile([128, NCT, 512], BF16, tag="xs", name=f"xs_{qc}")
        xs_eng.dma_start(
            out=xsc,
            in_=xT[:, qc * 512:(qc + 1) * 512].rearrange(
                "(c p) q -> p c q", p=128),
        )
        xs_tiles = [xsc[:, c, :] for c in range(NCT)]
        for _ in range(7 if qc > 0 else 1):
            yield
        qts = []
        for p in range(HPC // 2):
            ps = ps_pool.tile([128, 512], F32, tag="ps", name=f"qps_{qc}_{p}")
            for c in range(NCT):
                nc.tensor.matmul(
                    ps,
                    wq_sb[:, c, p * 128:(p + 1) * 128],
                    xs_tiles[c],
                    start=(c == 0),
                    stop=(c == NCT - 1),
                )
            qt = qt_pool.tile([128, 512], BF16, tag="qt", name=f"qt_{qc}_{p}")
            nc.vector.tensor_copy(qt, ps)
            qts.append(qt)
            yield
        qts_store[qc] = qts
        for p in range(HPC // 2):
            ps = ps_pool.tile([128, 512], F32, tag="ps", name=f"kps_{qc}_{p}")
            for c in range(NCT):
                nc.tensor.matmul(
                    ps,
                    wq_sb[:, c, FPC + p * 128:FPC + (p + 1) * 128],
                    xs_tiles[c],
                    start=(c == 0),
                    stop=(c == NCT - 1),
                )
            nc.vector.tensor_copy(kt[:, p, qc * 512:(qc + 1) * 512], ps)
            yield
        for j in range(4):
            kt_i = qc * 4 + j
            ps = ps_pool.tile([128, 512], F32, tag="ps", name=f"vps_{qc}_{j}")
            for c in range(NCT):
                nc.tensor.matmul(
                    ps,
                    xs_tiles[c][:, j * 128:(j + 1) * 128],
                    wq_sb[:, c, 2 * FPC:3 * FPC],
                    start=(c == 0),
                    stop=(c == NCT - 1),
                )
            nc.vector.tensor_copy(
                v2[:, kt_i, :, 0:HD],
                ps.rearrange("p (h d) -> p h d", h=HPC),
            )
            yield

    def gen_norm(qc, p):
        """Deferred denominator chain for head pair (qc, p): DMA-bridge the
        den rows (partition 64) to partition 0, reciprocal there (DVE),
        partition-broadcast to rows 0..63 (gpsimd, SBUF-to-SBUF), base-0
        stack muls (gpsimd).  Pulled 2+ pairs after the pair finished, so
        nothing ever stalls."""
        tmpA, tmpB = tmps_store[(qc, p)]
        nc.sync.dma_start(out=dn[0:1, 0:512], in_=tmpA[HD:HD + 1, :])
        nc.sync.dma_start(out=dn[0:1, 512:1024], in_=tmpB[HD:HD + 1, :])
        yield
        sc = scale_pool.tile([64, 1024], F32, tag="scale", name=f"sc_{qc}_{p}")
        nc.vector.reciprocal_approx_fast(sc[0:1, :], dn[0:1, :])
        yield
        nc.gpsimd.partition_broadcast(sc[0:64, 0:512], sc[0:1, 0:512],
                                      channels=64)
        nc.gpsimd.partition_broadcast(sc[0:64, 512:1024], sc[0:1, 512:1024],
                                      channels=64)
        yield
        stack = stack_pool.tile([128, 512], BF16, tag="stack", name=f"stk_{qc}_{p}")
        nc.vector.tensor_mul(stack[0:64, :], tmpA[0:64, :], sc[0:64, 0:512])
        yield
        stkB = stkb_pool.tile([64, 512], BF16, tag="stkB", name=f"skB_{qc}_{p}")
        nc.vector.tensor_mul(stkB[0:64, :], tmpB[0:64, :], sc[0:64, 512:1024])
        nc.sync.dma_start(out=stack[64:128, :], in_=stkB[0:64, :])
        stacks_store[qc][p] = stack
        yield

    def gen_proj(qc):
        stacks = stacks_store[qc]
        for tq in range(4):
            row0 = qc * 512 + tq * 128
            for ch in range(2):
                ps = ps_pool.tile([128, 512], F32, tag="ps",
                                  name=f"pps_{qc}_{tq}_{ch}")
                for p in range(HPC // 2):
                    nc.tensor.matmul(
                        ps,
                        stacks[p][:, tq * 128:(tq + 1) * 128],
                        wp_sb[:, p, ch * 512:(ch + 1) * 512],
                        start=(p == 0),
                        stop=(p == HPC // 2 - 1),
                    )
                pout = pout_pool.tile([128, 512], F32, tag="pout",
                                      name=f"po_{qc}_{tq}_{ch}")
                nc.vector.tensor_copy(pout, ps)
                # alternate DMA queue to break the pout-copy <-> out-DMA
                # recycle round-robin
                dma_eng = nc.sync if ch == 0 else nc.gpsimd
                dma_eng.dma_start(
                    out=out_ap[row0:row0 + 128, ch * 512:(ch + 1) * 512],
                    in_=pout,
                )
                yield

    def emit_tail_proj():
        """Last chunk's projection: all 8 groups' p=0..2 accumulations are
        emitted FIRST (they only need stacks 0..2, so they overlap the final
        norm chain on the PE), then the p=3 matmuls, then the evacuations.
        Eight PSUM banks: 2 ps + 2 y + halves of 2 st tiles."""
        qc = NQC - 1
        stacks = stacks_store[qc]
        groups = []
        st_tiles = []
        for tq in range(4):
            for ch in range(2):
                gi = tq * 2 + ch
                if gi < 2:
                    ps = ps_pool.tile([128, 512], F32, tag="ps",
                                      name=f"tp_ps_{gi}")
                elif gi < 4:
                    ps = y_pool.tile([128, 512], F32, tag="y",
                                     name=f"tp_y_{gi}")
                else:
                    if gi % 2 == 0:
                        st_tiles.append(st_pool.tile(
                            [128, 1024], F32, tag="st", name=f"tp_st_{gi}"))
                    half = st_tiles[-1]
                    ps = half[:, 0:512] if gi % 2 == 0 else half[:, 512:1024]
                for p in range(3):
                    nc.tensor.matmul(
                        ps,
                        stacks[p][:, tq * 128:(tq + 1) * 128],
                        wp_sb[:, p, ch * 512:(ch + 1) * 512],
                        start=(p == 0),
                        stop=False,
                    )
                groups.append((ps, tq, ch))
        for ps, tq, ch in groups:
            nc.tensor.matmul(
                ps,
                stacks[3][:, tq * 128:(tq + 1) * 128],
                wp_sb[:, 3, ch * 512:(ch + 1) * 512],
                start=False,
                stop=True,
            )
        for gi, (ps, tq, ch) in enumerate(groups):
            row0 = qc * 512 + tq * 128
            pout = pout_pool.tile([128, 512], F32, tag="pout",
                                  name=f"tp_po_{tq}_{ch}")
            if gi % 2 == 0:
                nc.vector.tensor_copy(pout, ps)
            else:
                nc.scalar.copy(pout, ps)
            # keep the tail out-writes off gpsimd so its (slow) ucode
            # drain starts immediately after the last broadcast
            dma_eng = nc.sync if ch == 0 else nc.scalar
            dma_eng.dma_start(
                out=out_ap[row0:row0 + 128, ch * 512:(ch + 1) * 512],
                in_=pout,
            )

    gen0 = gen_chunk(0)
    next(gen0)          # x(0) loads onto the scalar queue first
    load_weights()      # then the weight blocks, split across both queues
    for _ in gen0:
        pass
    # cast emitted after chunk-0 prep so it can't head-of-line block the
    # early qt/kt evacuations in the DVE FIFO
    nc.vector.tensor_copy(tri_bf, tri)

    fillers = []

    def pull(n):
        for _ in range(n):
            while fillers:
                try:
                    next(fillers[0])
                    break
                except StopIteration:
                    fillers.pop(0)

    for qc in range(NQC):
        stacks_store[qc] = [None] * 4
        if qc > 0:
            fillers.append(gen_norm(qc - 1, 2))
            fillers.append(gen_norm(qc - 1, 3))
        if qc + 1 < NQC:
            fillers.append(gen_chunk(qc + 1))
        if qc > 0:
            fillers.append(gen_proj(qc - 1))

        # ---- attention for this query chunk, per head pair ----
        nki = 4 * qc + 4
        qts = qts_store[qc]
        for p in range(HPC // 2):
            qt = qts[p]
            yA = y_pool.tile([128, 512], F32, tag="y", name=f"yA_{qc}_{p}")
            yB = y_pool.tile([128, 512], F32, tag="y", name=f"yB_{qc}_{p}")
            for ki in range(nki):
                r = ki - 4 * qc  # >= 0 on the block diagonal
                n0 = 128 * r if r > 0 else 0
                st = st_pool.tile([128, 1024], F32, tag="st",
                                  name=f"st_{qc}_{p}_{ki}")
                st3 = st.rearrange("p (h q) -> p h q", h=2)
                kslice = slice(ki * 128, (ki + 1) * 128)
                nc.tensor.matmul(
                    st3[:, 0, n0:512], kt[0:64, p, kslice], qt[0:64, n0:512],
                    start=True, stop=True,
                )
                nc.tensor.matmul(
                    st3[:, 1, n0:512], kt[64:128, p, kslice], qt[64:128, n0:512],
                    start=True, stop=True,
                )
                se = se_pool.tile([128, 1024], BF16, tag="se",
                                  name=f"se_{qc}_{p}_{ki}")
                se3 = se.rearrange("p (h q) -> p h q", h=2)
                nc.scalar.activation(
                    se3[:, :, n0:512], st3[:, :, n0:512], Exp,
                    bias=adj_sb[:, ki:ki + 1], scale=SCALE,
                )
                if r >= 0:
                    nc.vector.tensor_mul(
                        se3[:, :, n0:n0 + 128], se3[:, :, n0:n0 + 128], tri_b
                    )
                nc.tensor.matmul(
                    yA[0:HD + 1, n0:512], v2[:, ki, 2 * p, :], se3[:, 0, n0:512],
                    start=(ki == 0), stop=(ki == nki - 1), skip_group_check=True,
                )
                nc.tensor.matmul(
                    yB[0:HD + 1, n0:512], v2[:, ki, 2 * p + 1, :], se3[:, 1, n0:512],
                    start=(ki == 0), stop=(ki == nki - 1), skip_group_check=True,
                )
                # cadence: spread the filler items over the chunk's ki slots
                if qc == 0:
                    pull(2)
                elif qc == 1:
                    pull(2 if ki % 2 == 0 else 1)
                elif qc == 2:
                    pull(1)
                else:
                    # reserve fillers for the ACT-bound back half of the
                    # last chunk, where the PE otherwise runs dry
                    pull(1 if (ki >= 8 and ki % 2 == 0) else 0)

            # evacuate y^T (+ row HD denominators); everything else is
            # deferred to gen_norm pulled >= 2 pairs later
            tmpA = tmp_pool.tile([65, 512], F32, tag="tmp", name=f"tmpA_{qc}_{p}")
            nc.vector.tensor_copy(tmpA[0:HD + 1, :], yA[0:HD + 1, :])
            tmpB = tmp_pool.tile([65, 512], F32, tag="tmp", name=f"tmpB_{qc}_{p}")
            nc.vector.tensor_copy(tmpB[0:HD + 1, :], yB[0:HD + 1, :])
            tmps_store[(qc, p)] = (tmpA, tmpB)
            if p < 2 or qc == NQC - 1:
                fillers.append(gen_norm(qc, p))
            pull(2)

        pull(1000)

    # ---- tail: last chunk's projection ----
    emit_tail_proj()


def make_nc():
    nc = bacc.Bacc("TRN2", target_bir_lowering=False, debug=False,
                   num_devices=NCORES)
    xT = nc.dram_tensor("xT", [C, T], BF16, kind="ExternalInput")
    wqkv = nc.dram_tensor("wqkv", [C, 3 * FPC], BF16, kind="ExternalInput")
    wproj = nc.dram_tensor("wproj", [FPC, C], BF16, kind="ExternalInput")
    adj = nc.dram_tensor("adj", [128, NKT], F32, kind="ExternalInput")
    out = nc.dram_tensor("out", [T, C], F32, kind="ExternalOutput")
    tri_np = np.triu(np.ones((128, 128), dtype=np.float32))
    tri_dram = nc.inline_tensor(tri_np, name="tri_const")
    with ExitStack() as ctx:
        tc = ctx.enter_context(tile.TileContext(nc))
        tc.ctx = ctx
        build(tc, out[:, :], xT[:, :], wqkv[:, :], wproj[:, :], adj[:],
              tri_dram)
    nc.compile()
    return nc


def shard_inputs(x, prev_probs, W_attn, W_proj):
    import ml_dtypes

    in_maps = []
    for core in range(NCORES):
        b, g = divmod(core, 2)
        xT = np.ascontiguousarray(x[b].T)
        wq = W_attn[:, g * FPC:(g + 1) * FPC]
        wk = W_attn[:, C + g * FPC:C + (g + 1) * FPC]
        wv = W_attn[:, 2 * C + g * FPC:2 * C + (g + 1) * FPC]
        wqkv = np.ascontiguousarray(np.concatenate([wq, wk, wv], axis=1))
        wproj = np.ascontiguousarray(W_proj[g * FPC:(g + 1) * FPC, :])
        adj = (-np.float32(EPS_BIAS)
               * np.log(prev_probs[b] + np.float32(1e-10))).astype(np.float32)
        # pre-shuffle to adj_sb layout: [key-within-tile (partition), tile]
        adj = np.ascontiguousarray(adj.reshape(NKT, 128).T)
        in_maps.append(
            {
                "xT": xT.astype(ml_dtypes.bfloat16),
                "wqkv": wqkv.astype(ml_dtypes.bfloat16),
                "wproj": wproj.astype(ml_dtypes.bfloat16),
                "adj": adj,
            }
        )
    return in_maps


_CACHED_NC = None


def kernel(x, prev_probs, W_attn, W_proj, trace=False, tmpdir=None):
    global _CACHED_NC
    from concourse.bass_utils import run_bass_kernel_spmd

    x = np.asarray(x, dtype=np.float32)
    prev_probs = np.asarray(prev_probs, dtype=np.float32)
    W_attn = np.asarray(W_attn, dtype=np.float32)
    W_proj = np.asarray(W_proj, dtype=np.float32)

    if _CACHED_NC is None:
        _CACHED_NC = make_nc()
    nc = _CACHED_NC

    in_maps = shard_inputs(x, prev_probs, W_attn, W_proj)
    res = run_bass_kernel_spmd(
        nc, in_maps, core_ids=list(range(NCORES)), trace=trace, tmpdir=tmpdir
    )
    parts = [r["out"] for r in res.results]
    out = np.empty((B, T, C), dtype=np.float32)
    for b in range(B):
        out[b] = parts[2 * b] + parts[2 * b + 1]
    kernel.last_results = res
    return out
